# revision 1
# baseline (speedup 1.0000x reference)
"""Multi-head self-attention (B=8, S=1024, D=768, H=12) on 8 trn2 cores.

Sharding: data-parallel over batch — core b computes attention for Q[b].
No collectives. Host pre-transposes Q (to X^T) and the weights (to W^T,
i.e. [d_in, d_out]) so every on-device matmul contracts over the
partition dim with zero on-device transposes; the device returns
ctx^T [768, 1024] which the host transposes back.

Device layout (per core):
  qt   [768,1024] = Q[b]^T                    (d_in on partitions)
  w*t  [768, 768] = W^T                        (d_in on partitions)
  qT/kT [768,1024] = (XW^T+b)^T                (d_out on partitions)
  v    packed [128, 8, 12*65]: per head 64 v-columns + a ones column
       (ones row makes the ctx matmul also emit the softmax denominator)
  scores^T [s_k, s_q] per head: K=64 matmul; exp via ACT (scale=1/8 fused)
  ctx^T accum over s_k chunks: [65, 512] PSUM; row 64 = sum(exp) = Z
  normalize: ctx^T * (1/Z), 1/Z replicated across partitions via a
  DRAM round-trip DMA (SBUF sources cannot partition-broadcast)
"""

import ml_dtypes
import numpy as np

import concourse.bass as bass
import concourse.mybir as mybir
import concourse.tile as tile
from concourse.bass_utils import run_bass_kernel_spmd

F32 = mybir.dt.float32
BF16 = mybir.dt.bfloat16

S = 1024
D = 768
H = 12
DK = 64
KC = D // 128   # 6 contraction chunks
MC = D // 128   # 6 output-row chunks
SC = S // 128   # 8 sequence chunks
NSQ = S // 512  # 2 query-column chunks
SCALE = 1.0 / np.sqrt(DK)
VROW = 65       # 64 v columns + 1 ones column per head
KGROUPS = [(0, 3), (3, 3), (6, 2)]   # s_k chunk groups for scores/exp
KC2G = {g0 + i: (g, i) for g, (g0, glen) in enumerate(KGROUPS)
        for i in range(glen)}


def _split_excess_waits(nc, max_waits=1):
    """This container's walrus encodes at most one sem-wait per
    instruction; spread extra waits onto EventSemaphore instructions."""
    for fn in nc.m.functions:
        for bb in fn.blocks:
            out = []
            for ins in bb.instructions:
                si = getattr(ins, "sync_info", None)
                ow = list(si.on_wait) if (si is not None and si.on_wait) else []
                if len(ow) > max_waits:
                    head, tail = ow[:-max_waits], ow[-max_waits:]
                    for j in range(0, len(head), max_waits):
                        ev = mybir.InstEventSemaphore(
                            name=f"evsplit-{ins.name}-{j}", ins=[], outs=[])
                        ev.engine = ins.engine
                        ev.sync_info = mybir.SyncInfo(
                            on_wait=head[j:j + max_waits], on_update=[])
                        out.append(ev)
                    ins.sync_info = mybir.SyncInfo(
                        on_wait=tail, on_update=list(si.on_update))
                out.append(ins)
            bb.instructions = out


def build_nc():
    nc = bass.Bass(trn_type="TRN2")

    qt = nc.dram_tensor("qt", [D, S], BF16, kind="ExternalInput").ap()
    wqt = nc.dram_tensor("wqt", [D, D], BF16, kind="ExternalInput").ap()
    wkt = nc.dram_tensor("wkt", [D, D], BF16, kind="ExternalInput").ap()
    wvt = nc.dram_tensor("wvt", [D, D], BF16, kind="ExternalInput").ap()
    bq = nc.dram_tensor("bq", [D], F32, kind="ExternalInput").ap()
    bk = nc.dram_tensor("bk", [D], F32, kind="ExternalInput").ap()
    bv = nc.dram_tensor("bv", [D], F32, kind="ExternalInput").ap()
    ctxt = nc.dram_tensor("ctxt", [D, S], F32, kind="ExternalOutput").ap()

    with tile.TileContext(nc) as tc:
        with (
            tc.tile_pool(name="singles", bufs=1) as singles,
            tc.tile_pool(name="psA", bufs=2, space="PSUM") as psA,
            tc.tile_pool(name="psP", bufs=2, space="PSUM") as psP,
            tc.tile_pool(name="expp", bufs=4) as expp,
            tc.tile_pool(name="ctop", bufs=4) as ctop,
            tc.tile_pool(name="recp", bufs=4) as recp,
            tc.tile_pool(name="recd", bufs=3, space="DRAM") as recd,
        ):
            # ---- persistent SBUF arrays --------------------------------
            qt_sb = singles.tile([128, KC, S], BF16)      # X^T
            wq_sb = singles.tile([128, KC, D], BF16)      # Wq^T
            wk_sb = singles.tile([128, KC, D], BF16)
            wv_sb = singles.tile([128, KC, D], BF16)
            qT_sb = singles.tile([128, MC, S], BF16)      # q^T
            kT_sb = singles.tile([128, MC, S], BF16)
            v_sb = singles.tile([128, SC, H * VROW], BF16)
            bq_sb = singles.tile([128, MC], F32)
            bk_sb = singles.tile([128, MC], F32)
            bvb_sb = singles.tile([128, H, DK], BF16)     # bv bcast over partitions

            # ---- input DMAs, ordered so q/k mc=0 projections start ASAP:
            # qt chunk-by-chunk, first column-slice of wq/wk, then the rest.
            nc.sync.dma_start(out=bq_sb, in_=bq.rearrange("(c p) -> p c", p=128))
            nc.scalar.dma_start(out=bk_sb,
                                in_=bk.rearrange("(c p) -> p c", p=128))
            qtr = qt.rearrange("(c p) s -> p c s", p=128)
            dma_engs = [nc.sync, nc.scalar]
            for kc in range(KC):
                dma_engs[kc % 2].dma_start(out=qt_sb[:, kc, :],
                                           in_=qtr[:, kc, :])
            wqr = wqt.rearrange("(c p) n -> p c n", p=128)
            wkr = wkt.rearrange("(c p) n -> p c n", p=128)
            nc.sync.dma_start(out=wq_sb[:, :, 0:128], in_=wqr[:, :, 0:128])
            nc.scalar.dma_start(out=wk_sb[:, :, 0:128], in_=wkr[:, :, 0:128])
            wvr = wvt.rearrange("(c p) n -> p c n", p=128)
            nc.sync.dma_start(out=wv_sb[:, :, 0:384], in_=wvr[:, :, 0:384])
            nc.scalar.dma_start(out=wv_sb[:, :, 384:768], in_=wvr[:, :, 384:768])
            nc.sync.dma_start(out=wq_sb[:, :, 128:768], in_=wqr[:, :, 128:768])
            nc.scalar.dma_start(out=wk_sb[:, :, 128:768], in_=wkr[:, :, 128:768])
            bv_bcast = bass.AP(tensor=bv.tensor, offset=bv.offset,
                               ap=[[0, 128], [DK, H], [1, DK]])
            nc.gpsimd.dma_start(out=bvb_sb, in_=bv_bcast)  # casts f32->bf16

            # ones columns of v (col 64 of each 65-wide head group)
            v4 = v_sb.rearrange("p s (h c) -> p s h c", c=VROW)
            nc.vector.memset(v4[:, :, :, DK:DK + 1], 1.0)

            # ---- emission helpers (emission order == scheduler priority) --

            def proj_qk(mc):
                """q^T and k^T rows for head pair mc."""
                for (w_sb, b_sb, o_sb) in ((wq_sb, bq_sb, qT_sb),
                                           (wk_sb, bk_sb, kT_sb)):
                    for n in range(NSQ):
                        ps = psP.tile([128, 512], F32, tag="proj",
                                      name=f"pj_{mc}_{n}")
                        for kc in range(KC):
                            nc.tensor.matmul(
                                ps,
                                lhsT=w_sb[:, kc, mc * 128:(mc + 1) * 128],
                                rhs=qt_sb[:, kc, n * 512:(n + 1) * 512],
                                start=(kc == 0), stop=(kc == KC - 1),
                            )
                        nc.vector.tensor_scalar_add(
                            out=o_sb[:, mc, n * 512:(n + 1) * 512],
                            in0=ps,
                            scalar1=b_sb[:, mc:mc + 1],
                        )

            def proj_v():
                """v[s, d] = X @ Wv^T + bv, packed 65-strided with ones col."""
                for sc in range(SC):
                    for n in range(2):       # d_out in two 384 chunks
                        ps = psP.tile([128, 512], F32, tag="proj",
                                      name=f"pv_{sc}_{n}")
                        for kc in range(KC):
                            nc.tensor.matmul(
                                ps[:, 0:384],
                                lhsT=qt_sb[:, kc, sc * 128:(sc + 1) * 128],
                                rhs=wv_sb[:, kc, n * 384:(n + 1) * 384],
                                start=(kc == 0), stop=(kc == KC - 1),
                            )
                        nc.vector.tensor_add(
                            out=v4[:, sc, 6 * n:6 * n + 6, 0:DK],
                            in0=ps[:, 0:384].rearrange("p (h c) -> p h c", c=DK),
                            in1=bvb_sb[:, 6 * n:6 * n + 6, :],
                        )

            def sc_exp(mc, j):
                """Scores^T + exp for both heads of pair mc, query cols j.
                The two heads' K=64 matmuls are emitted back-to-back from
                partition bases 0/64 so the PE runs them concurrently in
                distinct row groups (row tiling)."""
                # kc groups of (3,3,2): bigger ACT instructions amortize
                # the per-instruction ACT overhead; one exp tile per
                # (head, group) so ctx matmuls unblock per group
                exp_ts = [[expp.tile([128, glen, 512], BF16,
                                     tag=f"exp{hh}g{g}",
                                     name=f"exp_{mc}_{j}_{hh}_{g}")
                           for g, (g0, glen) in enumerate(KGROUPS)]
                          for hh in range(2)]
                for g, (g0, glen) in enumerate(KGROUPS):
                    # both heads draw from one 2-slot rotation (6 banks):
                    # ACT drains slot A while the PE refills slot B
                    pss = [psA.tile([128, 3, 512], F32, tag="sc",
                                    name=f"sc_{mc}_{j}_{g}_{hh}")
                           for hh in range(2)]
                    for i in range(glen):
                        kc2 = g0 + i
                        for hh in range(2):
                            pb = hh * DK
                            nc.tensor.matmul(
                                pss[hh][:, i, :],
                                lhsT=kT_sb[pb:pb + DK, mc,
                                           kc2 * 128:(kc2 + 1) * 128],
                                rhs=qT_sb[pb:pb + DK, mc,
                                          j * 512:(j + 1) * 512],
                                start=True, stop=True,
                            )
                    for hh in range(2):
                        nc.scalar.activation(
                            out=exp_ts[hh][g],
                            in_=pss[hh][:, 0:glen, :],
                            func=mybir.ActivationFunctionType.Exp,
                            scale=float(SCALE),
                        )
                return exp_ts

            def ctx(mc, j, exp_ts):
                """ctx^T + normalization for both heads of pair mc."""
                for hh in range(2):
                    h = 2 * mc + hh
                    exp_t = exp_ts[hh]
                    # ctx^T (rows 0:64) + Z (row 64), accumulated over s_k
                    psc = psP.tile([128, 512], F32, tag="proj",
                                   name=f"psc_{mc}_{j}_{hh}")
                    for kc2 in range(SC):
                        g, i = KC2G[kc2]
                        nc.tensor.matmul(
                            psc[0:VROW, :],
                            lhsT=v_sb[:, kc2, h * VROW:(h + 1) * VROW],
                            rhs=exp_t[g][:, i, :],
                            start=(kc2 == 0), stop=(kc2 == SC - 1),
                        )
                    # free the PSUM bank quickly: copy ctx+Z to SBUF, then
                    # run the recip/broadcast/normalize chain off SBUF
                    cts = ctop.tile([VROW, 512], F32, tag="cts",
                                    name=f"cts_{mc}_{j}_{hh}")
                    nc.vector.tensor_copy(out=cts, in_=psc[0:VROW, :])
                    rec = recp.tile([1, 512], F32, tag="rec",
                                    name=f"rec_{mc}_{j}_{hh}")
                    nc.vector.reciprocal(out=rec, in_=cts[DK:DK + 1, :])
                    # SBUF->SBUF partition-broadcast is not allowed; bounce
                    # the 2KB row through DRAM and read it back replicated
                    # across 64 partitions.
                    recdram = recd.tile([1, 512], F32, tag="recd",
                                        name=f"recd_{mc}_{j}_{hh}")
                    nc.sync.dma_start(out=recdram, in_=rec)
                    recb = recp.tile([64, 512], F32, tag="recb",
                                     name=f"recb_{mc}_{j}_{hh}")
                    nc.sync.dma_start(out=recb,
                                      in_=recdram.to_broadcast([64, 512]))
                    cto = ctop.tile([64, 512], F32, tag="cto",
                                    name=f"cto_{mc}_{j}_{hh}")
                    nc.vector.tensor_mul(out=cto, in0=cts[0:DK, :], in1=recb)
                    nc.sync.dma_start(
                        out=ctxt[h * DK:(h + 1) * DK, j * 512:(j + 1) * 512],
                        in_=cto)

            # ---- software pipeline ----------------------------------------
            # qk(0) + scores/exp(0) first so ACT starts ~5us in; v overlaps
            # pair-0 exps; from then on ctx(mc) is deferred past qk(mc+1) and
            # sc_exp(mc+1) emission so the PE always has high-priority work
            # while ACT chews the previous pair's exps.
            proj_qk(0)
            exps = {j: sc_exp(0, j) for j in range(NSQ)}
            proj_v()
            for mc in range(1, MC):
                proj_qk(mc)
                for j in range(NSQ):
                    ctx(mc - 1, j, exps[j])
                if mc < MC - 1:
                    exps = {j: sc_exp(mc, j) for j in range(NSQ)}
                else:
                    for j in range(NSQ):
                        e = sc_exp(mc, j)
                        ctx(mc, j, e)

    _split_excess_waits(nc)
    return nc


_NC_CACHE = None


def _get_nc():
    global _NC_CACHE
    if _NC_CACHE is None:
        _NC_CACHE = build_nc()
    return _NC_CACHE


def kernel(Q, Wq, bq, Wk, bk, Wv, bv):
    BF = ml_dtypes.bfloat16
    Q = np.asarray(Q, np.float32)
    wqt = np.ascontiguousarray(np.asarray(Wq, np.float32).T.astype(BF))
    wkt = np.ascontiguousarray(np.asarray(Wk, np.float32).T.astype(BF))
    wvt = np.ascontiguousarray(np.asarray(Wv, np.float32).T.astype(BF))
    bq = np.ascontiguousarray(np.asarray(bq, np.float32))
    bk = np.ascontiguousarray(np.asarray(bk, np.float32))
    bv = np.ascontiguousarray(np.asarray(bv, np.float32))

    nc = _get_nc()
    in_maps = []
    for b in range(Q.shape[0]):
        in_maps.append({
            "qt": np.ascontiguousarray(Q[b].T.astype(BF)),
            "wqt": wqt, "wkt": wkt, "wvt": wvt,
            "bq": bq, "bk": bk, "bv": bv,
        })
    res = run_bass_kernel_spmd(nc, in_maps, core_ids=list(range(len(in_maps))))
    out = np.stack([np.ascontiguousarray(r["ctxt"].T) for r in res.results])
    return out



# revision 2
# speedup vs baseline: 1.1660x; 1.1660x over previous
"""Multi-head self-attention (B=8, S=1024, D=768, H=12) on 8 trn2 cores.

Data-parallel over batch: core b computes full attention for Q[b]; no
collectives.

Per-core pipeline (all matmul compute on device):
  projections  hi/lo-fp8 DoubleRow: X and W are split on the host into
               fp8e4m3 hi + residual-lo pairs (power-of-2 pre-scales keep
               both in the fp8 normal range). Each 128-deep contraction
               chunk takes 2 DR matmuls:
                 (Wh,Wh).T@(Xh,Xl) + (Wl,Wl).T@(Xh,Xl) = W.T@X   (exact-ish)
               at 0.5 cycles/row -> 3x cheaper than fp32 per matrix.
  scores       float32r matmuls on full-precision q^T/k^T (d_k on
               partitions), accurate to ~1e-4.
  exp          split between ACT (true Exp, scale=1/8 fused) and DVE
               (Schraudolph fast-exp: one tensor_scalar into int16,
               bitcast to bf16; ~±3% on the affected softmax weights).
  ctx          bf16, [q,65] orientation: lhsT=exp^T chunk [k,128q],
               rhs=v[k,64+ones] -> psum [q, 64 ctx | Z]; the ones column
               accumulates the softmax denominator Z.
  output       raw ctx+Z are DMA'd out; the host divides by Z, adds bv
               (exact: softmax rows sum to 1), and reshapes.
  biases       bk is dropped exactly (constant shift per query row is
               softmax-invariant); bq is fused into the q psum->sbuf
               conversion; bv is added on the host.
"""

import ml_dtypes
import numpy as np

import concourse.bass as bass
import concourse.mybir as mybir
import concourse.tile as tile
from concourse.bass_utils import run_bass_kernel_spmd

F32 = mybir.dt.float32
F32R = mybir.dt.float32r
BF16 = mybir.dt.bfloat16
FP8 = mybir.dt.float8e4
I16 = mybir.dt.int16
DR = mybir.MatmulPerfMode.DoubleRow

S = 1024
D = 768
H = 12
DK = 64
KC = D // 128        # 6 contraction chunks of 128
MC = D // 128        # 6 output-row chunks (d_out)
SC = S // 128        # 8 sequence chunks
VROW = DK + 1        # 64 v columns + ones column
X_SCALE = 8.0        # host pre-scale on X (power of 2; keeps fp8 normal)
W_SCALE = 64.0       # host pre-scale on W^T
PSUM_SCALE = 1.0 / (X_SCALE * W_SCALE)   # 1/512, exact
A16 = 128.0 / np.log(2.0)                # schraudolph slope (int16->bf16)
FAST_BIAS = 16256.0 - 5.0                # bf16 exponent bias + centering
SCORE_SCALE = 0.125                      # 1/sqrt(64)

# exp engine map per head: 1 = ACT true exp, 0 = DVE fast exp. 8 units per
# head (j in {0,1} x t in {0..3}); even/odd heads alternate 5/3 and 4/4.
EXP_PAT_EVEN = (1, 0, 1, 1, 0, 1, 0, 1)
EXP_PAT_ODD = (1, 0, 1, 0, 1, 1, 0, 0)


def _split_excess_waits(nc, max_waits=1):
    """This container's walrus encodes at most one sem-wait per
    instruction; spread extra waits onto EventSemaphore instructions."""
    for fn in nc.m.functions:
        for bb in fn.blocks:
            out = []
            for ins in bb.instructions:
                si = getattr(ins, "sync_info", None)
                ow = list(si.on_wait) if (si is not None and si.on_wait) else []
                if len(ow) > max_waits:
                    head, tail = ow[:-max_waits], ow[-max_waits:]
                    for j in range(0, len(head), max_waits):
                        ev = mybir.InstEventSemaphore(
                            name=f"evsplit-{ins.name}-{j}", ins=[], outs=[])
                        ev.engine = ins.engine
                        ev.sync_info = mybir.SyncInfo(
                            on_wait=head[j:j + max_waits], on_update=[])
                        out.append(ev)
                    ins.sync_info = mybir.SyncInfo(
                        on_wait=tail, on_update=list(si.on_update))
                out.append(ins)
            bb.instructions = out


def build_nc():
    nc = bass.Bass(trn_type="TRN2")

    # X^T hi/lo pairs, one dram tensor per 128-contraction chunk:
    # [128, 2, 1024] with slot dim = (Xh, Xl)
    x2 = [nc.dram_tensor(f"x2_{c}", [128, 2, S], FP8,
                         kind="ExternalInput").ap() for c in range(KC)]
    # weights: per matrix and hi/lo-dup, 6 d_out-chunk tensors
    # [128, KC, 2, 128]; slot dim duplicates (Wh,Wh) / (Wl,Wl)
    wq = {hl: [nc.dram_tensor(f"wq{hl}_{n}", [128, KC, 2, 128], FP8,
                              kind="ExternalInput").ap() for n in range(6)]
          for hl in ("h", "l")}
    wk = {hl: [nc.dram_tensor(f"wk{hl}_{n}", [128, KC, 2, 128], FP8,
                              kind="ExternalInput").ap() for n in range(6)]
          for hl in ("h", "l")}
    # v weights as two 384-wide moving halves
    wv = {hl: [nc.dram_tensor(f"wv{hl}_{n}", [128, KC, 2, 384], FP8,
                              kind="ExternalInput").ap() for n in range(2)]
          for hl in ("h", "l")}
    bq = nc.dram_tensor("bq", [D], F32, kind="ExternalInput").ap()
    # output: raw ctx (64 cols) + Z (col 64) per head, [S, H, 65] f32
    ctxo = nc.dram_tensor("ctxo", [S, H, VROW], F32,
                          kind="ExternalOutput").ap()

    with tile.TileContext(nc) as tc:
        with (
            tc.tile_pool(name="singles", bufs=1) as singles,
            tc.tile_pool(name="psA", bufs=2, space="PSUM") as psA,    # proj+ctx
            tc.tile_pool(name="psS", bufs=3, space="PSUM") as psS,    # scores
            tc.tile_pool(name="expA", bufs=10) as expA,
            tc.tile_pool(name="expD", bufs=10) as expD,
        ):
            x2_sb = [singles.tile([128, 2, S], FP8, name=f"x2sb{c}")
                     for c in range(KC)]
            wq_sb = {hl: [singles.tile([128, KC, 2, 128], FP8,
                                       name=f"wq{hl}{n}") for n in range(6)]
                     for hl in ("h", "l")}
            wk_sb = {hl: [singles.tile([128, KC, 2, 128], FP8,
                                       name=f"wk{hl}{n}") for n in range(6)]
                     for hl in ("h", "l")}
            wv_sb = {hl: [singles.tile([128, KC, 2, 384], FP8,
                                       name=f"wv{hl}{n}") for n in range(2)]
                     for hl in ("h", "l")}
            qT_sb = singles.tile([128, MC, S], F32R)
            kT_sb = singles.tile([128, MC, S], F32R)
            v4 = singles.tile([128, SC, H, VROW], BF16)
            bq_sb = singles.tile([128, MC], F32)
            # ctx staging: 3 head-quad tiles [128, qc, 4 heads, 65] f32
            ctx_sb = [singles.tile([128, SC, 4, VROW], F32, name=f"ctxsb{g}")
                      for g in range(3)]

            # ---- input DMAs (sync queue), ordered for proj start ---------
            nc.sync.dma_start(out=bq_sb,
                              in_=bq.rearrange("(c p) -> p c", p=128))
            nc.sync.dma_start(out=wq_sb["h"][0], in_=wq["h"][0])
            nc.sync.dma_start(out=wq_sb["l"][0], in_=wq["l"][0])
            for c in range(KC):
                nc.sync.dma_start(out=x2_sb[c], in_=x2[c])
            for n in range(1, 6):
                nc.sync.dma_start(out=wq_sb["h"][n], in_=wq["h"][n])
                nc.sync.dma_start(out=wq_sb["l"][n], in_=wq["l"][n])
            for n in range(6):
                nc.sync.dma_start(out=wk_sb["h"][n], in_=wk["h"][n])
                nc.sync.dma_start(out=wk_sb["l"][n], in_=wk["l"][n])
            for n in range(2):
                nc.sync.dma_start(out=wv_sb["h"][n], in_=wv["h"][n])
                nc.sync.dma_start(out=wv_sb["l"][n], in_=wv["l"][n])

            nc.vector.memset(v4[:, :, :, DK:VROW], 1.0)

            # ---- emission helpers ---------------------------------------

            def proj_qk(mc):
                """q^T and k^T rows [mc*128, +128) for all of S."""
                for (w_sb, o_sb, bias) in ((wq_sb, qT_sb, True),
                                           (wk_sb, kT_sb, False)):
                    for n in range(2):  # two 512-col halves of S
                        ps = psA.tile([128, 512], F32, tag="pj",
                                      name=f"pj_{mc}_{n}")
                        for c in range(KC):
                            for hl in ("h", "l"):
                                nc.tensor.matmul(
                                    ps,
                                    lhsT=w_sb[hl][mc][:, c, :, :],
                                    rhs=x2_sb[c][:, :, n * 512:(n + 1) * 512],
                                    start=(c == 0 and hl == "h"),
                                    stop=(c == KC - 1 and hl == "l"),
                                    perf_mode=DR,
                                )
                        if bias:
                            nc.vector.tensor_scalar(
                                out=o_sb[:, mc, n * 512:(n + 1) * 512],
                                in0=ps,
                                scalar1=PSUM_SCALE,
                                scalar2=bq_sb[:, mc:mc + 1],
                                op0=mybir.AluOpType.mult,
                                op1=mybir.AluOpType.add,
                            )
                        else:
                            nc.vector.tensor_scalar_mul(
                                out=o_sb[:, mc, n * 512:(n + 1) * 512],
                                in0=ps,
                                scalar1=PSUM_SCALE,
                            )

            def proj_v():
                """v[s, d] via swapped operands: lhsT=X chunk, rhs=Wv."""
                for sc in range(SC):
                    for n in range(2):  # d_out in two 384-wide halves
                        ps = psA.tile([128, 512], F32, tag="pj",
                                      name=f"pv_{sc}_{n}")
                        for c in range(KC):
                            for hl in ("h", "l"):
                                nc.tensor.matmul(
                                    ps[:, 0:384],
                                    lhsT=x2_sb[c][:, :,
                                                  sc * 128:(sc + 1) * 128],
                                    rhs=wv_sb[hl][n][:, c, :, :],
                                    start=(c == 0 and hl == "h"),
                                    stop=(c == KC - 1 and hl == "l"),
                                    perf_mode=DR,
                                )
                        # scale to natural units, cast bf16, scatter per head
                        nc.scalar.mul(
                            out=v4[:, sc, 6 * n:6 * n + 6, 0:DK],
                            in_=ps[:, 0:384].rearrange("p (h c) -> p h c",
                                                       c=DK),
                            mul=PSUM_SCALE,
                        )

            def scores_exp(h):
                """Scores + exp for head h; returns 8 exp tiles keyed
                (j, t) each [128, 2, 512] bf16-readable."""
                mc = h // 2
                pb = (h % 2) * DK
                pat = EXP_PAT_EVEN if h % 2 == 0 else EXP_PAT_ODD
                out = {}
                for j in range(2):
                    for t in range(4):
                        ps = psS.tile([128, 2, 512], F32, tag="sc",
                                      name=f"sc_{h}_{j}_{t}")
                        for i in range(2):
                            kc = 2 * t + i
                            nc.tensor.matmul(
                                ps[:, i, :],
                                lhsT=kT_sb[pb:pb + DK, mc,
                                           kc * 128:(kc + 1) * 128],
                                rhs=qT_sb[pb:pb + DK, mc,
                                          j * 512:(j + 1) * 512],
                                start=True, stop=True,
                            )
                        if pat[j * 4 + t]:
                            e = expA.tile([128, 2, 512], BF16, tag="eA",
                                          name=f"eA_{h}_{j}_{t}")
                            nc.scalar.activation(
                                out=e, in_=ps,
                                func=mybir.ActivationFunctionType.Exp,
                                scale=SCORE_SCALE,
                            )
                            out[(j, t)] = e
                        else:
                            e = expD.tile([128, 2, 512], I16, tag="eD",
                                          name=f"eD_{h}_{j}_{t}")
                            nc.vector.tensor_scalar(
                                out=e, in0=ps,
                                scalar1=float(SCORE_SCALE * A16),
                                scalar2=float(FAST_BIAS),
                                op0=mybir.AluOpType.mult,
                                op1=mybir.AluOpType.add,
                            )
                            out[(j, t)] = e.bitcast(BF16)
                return out

            def ctx(h, exp_ts):
                """ctx+Z for head h: per j a [128, 4, 65] psum, 4 q-chains
                of 8 accumulation steps; copy into ctx staging."""
                for j in range(2):
                    ps = psA.tile([128, 4, VROW], F32, tag="pj",
                                  name=f"pc_{h}_{j}")
                    for qi in range(4):
                        for kc in range(SC):
                            e = exp_ts[(j, kc // 2)]
                            nc.tensor.matmul(
                                ps[:, qi, :],
                                lhsT=e[:, kc % 2,
                                       qi * 128:(qi + 1) * 128],
                                rhs=v4[:, kc, h, :],
                                start=(kc == 0), stop=(kc == SC - 1),
                            )
                    dst = ctx_sb[h // 4][:, j * 4:(j + 1) * 4, h % 4, :]
                    if h % 2 == 0:
                        nc.scalar.copy(out=dst, in_=ps)
                    else:
                        nc.vector.tensor_copy(out=dst, in_=ps)

            def out_dma(g):
                """DMA head-quad g for all 8 q-chunks."""
                for qc in range(SC):
                    nc.sync.dma_start(
                        out=ctxo[qc * 128:(qc + 1) * 128,
                                 4 * g:4 * (g + 1), :],
                        in_=ctx_sb[g][:, qc, :, :],
                    )

            # ---- schedule -----------------------------------------------
            for mc in range(MC):
                proj_qk(mc)
            proj_v()

            prev = None
            for h in range(H):
                exp_ts = scores_exp(h)
                if prev is not None:
                    ctx(*prev)
                    if prev[0] == 3:
                        out_dma(0)
                    elif prev[0] == 7:
                        out_dma(1)
                prev = (h, exp_ts)
            ctx(*prev)
            out_dma(2)

    _split_excess_waits(nc)
    return nc


_NC_CACHE = None


def _get_nc():
    global _NC_CACHE
    if _NC_CACHE is None:
        _NC_CACHE = build_nc()
    return _NC_CACHE


def _hi_lo(a):
    """fp8e4m3 hi + residual-lo split (values pre-scaled to normal range)."""
    F8 = ml_dtypes.float8_e4m3
    hi = a.astype(F8)
    lo = (a - hi.astype(np.float32)).astype(F8)
    return hi, lo


def kernel(Q, Wq, bq, Wk, bk, Wv, bv):
    Q = np.asarray(Q, np.float32)
    Wq = np.asarray(Wq, np.float32)
    Wk = np.asarray(Wk, np.float32)
    Wv = np.asarray(Wv, np.float32)
    bq = np.ascontiguousarray(np.asarray(bq, np.float32))
    bv = np.asarray(bv, np.float32)
    B = Q.shape[0]

    # weights: W^T scaled, hi/lo split, DR layouts (shared across cores)
    def prep_qk(W):
        wt = W.T * W_SCALE                       # [d_in, d_out]
        hi, lo = _hi_lo(wt)
        out = {}
        for hl, w8 in (("h", hi), ("l", lo)):
            # [d_in, d_out] -> per d_out chunk n: [128, KC, 2, 128] dup slots
            w4 = np.ascontiguousarray(
                w8.reshape(KC, 128, D).transpose(1, 0, 2))  # [128, KC, D]
            w4 = np.repeat(w4[:, :, None, :], 2, axis=2)    # [128, KC, 2, D]
            out[hl] = [np.ascontiguousarray(w4[:, :, :, n * 128:(n + 1) * 128])
                       for n in range(6)]
        return out

    def prep_v(W):
        wt = W.T * W_SCALE
        hi, lo = _hi_lo(wt)
        out = {}
        for hl, w8 in (("h", hi), ("l", lo)):
            w4 = np.ascontiguousarray(
                w8.reshape(KC, 128, D).transpose(1, 0, 2))
            w4 = np.repeat(w4[:, :, None, :], 2, axis=2)
            out[hl] = [np.ascontiguousarray(w4[:, :, :, n * 384:(n + 1) * 384])
                       for n in range(2)]
        return out

    wq_np = prep_qk(Wq)
    wk_np = prep_qk(Wk)
    wv_np = prep_v(Wv)

    nc = _get_nc()
    in_maps = []
    for b in range(B):
        xt = np.ascontiguousarray(Q[b].T) * X_SCALE     # [D, S]
        xh, xl = _hi_lo(xt)
        m = {"bq": bq}
        for c in range(KC):
            m[f"x2_{c}"] = np.ascontiguousarray(
                np.stack([xh[c * 128:(c + 1) * 128],
                          xl[c * 128:(c + 1) * 128]], axis=1))
        for n in range(6):
            m[f"wqh_{n}"] = wq_np["h"][n]
            m[f"wql_{n}"] = wq_np["l"][n]
            m[f"wkh_{n}"] = wk_np["h"][n]
            m[f"wkl_{n}"] = wk_np["l"][n]
        for n in range(2):
            m[f"wvh_{n}"] = wv_np["h"][n]
            m[f"wvl_{n}"] = wv_np["l"][n]
        in_maps.append(m)

    res = run_bass_kernel_spmd(nc, in_maps, core_ids=list(range(B)))
    out = np.empty((B, S, D), np.float32)
    for b in range(B):
        r = res.results[b]["ctxo"]          # [S, H, 65]
        ctx_raw = r[:, :, :DK]
        z = r[:, :, DK]
        out[b] = (ctx_raw / z[:, :, None]).reshape(S, D)
    out += bv
    return out
